# revision 30
# baseline (speedup 1.0000x reference)
"""Multi-head attention (bs=2, heads=8, ch=64, len=4096) on 8 Trainium2 cores.

Sharding: bs*heads = 16 head-problems, 2 per core (head/data parallel,
no cross-core communication).

Per-core algorithm (per head, seq len L=4096, ch=64):
  - S^T tiles: for s-tile j (128 rows) and t-chunk i (512 cols):
        st[s,t] = sum_c K[c,s] Q[c,t]         (PE, fp16, K=64)
    Head 0 uses array rows 0-63, head 1 rows 64-127 (tile_position),
    so adjacent A/B QK matmuls overlap in the array (~2x).
    q,k are prescaled by sqrt(1024*log2(e)*ch^-0.5) at cast time, so the
    PSUM logits arrive as Z = 1024*log2(e)*scale*logit.
  - expS^T: softmax numerator tiles in fp16, computed on TWO engines to
    split the 33.5M-exp/core wall (ScalarE alone = 1 elem/lane/cycle
    @1.2GHz = ~300us):
      * ScalarE windows: ACT exp with scale=ln2/1024, bias=ln2/2
        -> sqrt(2)*exp(scale*logit) in fp16.
      * DVE windows: custom 8-slice DVE op EXP2_FP16_ANT writing int16
        fp16-bits directly: magic-add floor split z=e+f, quadratic
        2^f-1 mantissa polynomial, +exponent bias, RNE int16 convert at
        the write port; bitcast fp16 = sqrt(2)*2^z (max rel err ~3e-3).
    Both paths carry the same global sqrt(2) factor, which cancels in
    the softmax normalize.
  - PV with folded denominator: lhsT = [V^T ; ones] ([128, 65] fp16,
    built once per head via PE transpose), accumulated over 32 s-tiles:
        pv[c,t]  = sum_s V[c,s] expS^T[s,t]   (c = 0..63)
        pv[64,t] = sum_s expS^T[s,t]          (softmax denominator)
  - normalize: o[c,t] = pv[c,t] * (1/pv[64,t])
    (custom-DVE fast reciprocal ~51ULP, GpSimd partition-broadcast,
    DVE multiply)

Structural notes (from NTFF profiling):
  - every TensorE instruction is chained in emission order
    (add_dep_helper sync=False): the Tile scheduler otherwise groups
    same-row-group K=64 matmuls, which serialize at 427ns (LDWEIGHTS
    cannot be pulled ahead); the emitted order runs warm with A/B QK
    pairs overlapping.
  - exp windows are 2 PSUM banks x 3 bufs so QK runs ahead of the two
    exp engines.
  - input DMAs are [128]-partition transfers with small starter tiles
    so compute begins at ~10us while the bulk streams.
"""

import os
import sys

sys.path.insert(0, "/opt/trn_rl_repo")

import math

import numpy as np
from concourse import mybir, tile, bacc, dve_ops
from concourse import bass_utils as _bu
from concourse.bass_utils import run_bass_kernel_spmd

if os.environ.get("KERNEL_LDW_OPT"):
    _orig_run_command = _bu.run_command

    def _run_command_ldw(cmd, *a, **k):
        cmd = [
            "--enable-ldw-opt=true" if c == "--enable-ldw-opt=false" else c
            for c in cmd
        ]
        return _orig_run_command(cmd, *a, **k)

    _bu.run_command = _run_command_ldw
from concourse.dve_spec import Spec, Src0, Src1, C0, C1, C2, lower
from concourse.dve_uop import DveOpSpec
from concourse.masks import make_identity
from concourse.tile_rust import add_dep_helper

dt = mybir.dt

NUM_HEADS = 8
BS = 2
CH = 64
L = 4096
NCORES = 8
HPC = BS * NUM_HEADS // NCORES  # heads per core = 2
NT = 512  # t-chunk (matmul moving dim / PSUM bank)
TCH = L // NT  # 8 t-chunks
NJ = L // 128  # 32 s-tiles
WSZ = 2  # s-tiles per exp window (2 PSUM banks; bufs=3 -> 6 banks)
SCALE = float(CH) ** -0.5

# --- exp split constants ----------------------------------------------------
LOG2E = math.log2(math.e)
QK_PRE = math.sqrt(1024.0 * LOG2E * SCALE)  # q,k cast-time prescale
ACT_SCALE = math.log(2.0) / 1024.0  # ScalarE: exp(Z*ln2/1024 + ln2/2)
ACT_BIAS = 0.5 * math.log(2.0)
# quadratic minimax fit of 2^g-1 on g=[0,1] (bitcast rel err <= 2.8e-3)
C0_POLY = 0.41470000
C1_POLY = 0.99176072
C2_POLY = 0.33573335
M_MAGIC = float(1.5 * 2**33)
G2 = C2_POLY / 1024.0
G1 = C1_POLY
OFF = 15360.0 + 1024.0 * C0_POLY
# of each 16 consecutive (window, head) exp tiles, this many go to the DVE
DVE_NUM = int(os.environ.get("KERNEL_DVE_NUM", "7"))
DVE_DEN = int(os.environ.get("KERNEL_DVE_DEN", "16"))

# --- custom DVE op: int16 fp16-bits of sqrt(2)*2^(Z/1024) -------------------
_t = Src0 + C0
_E = _t - C0
_F = Src0 - _E
_u = (_F * C1 + C2) * _F
_EXP2_BODY = (_E + _u) + Src1


def _exp2_ref(in0, in1, s0, s1, imm2):
    z = in0.astype(np.float32)
    tt = (z + np.float32(s0)).astype(np.float32)
    Ee = (tt - np.float32(s0)).astype(np.float32)
    Ff = (z - Ee).astype(np.float32)
    uu = ((Ff * np.float32(s1) + np.float32(imm2)) * Ff).astype(np.float32)
    return (Ee + uu) + in1


def _make_exp2_op():
    name = "EXP2_FP16_ANT"
    for op in dve_ops.OPS:
        if op.name == name:
            return op
    spec = Spec(body=_EXP2_BODY, reference=_exp2_ref)
    row = dve_ops._CUSTOM_DVE_ROW_BASE + len(dve_ops.OPS)
    sha = DveOpSpec(
        name=name, opcode=row, uops=lower(spec, ver="v3"), rd1_en=True
    ).sha("v3")
    op = dve_ops.DveOp(name, spec, subdim=False, uops_sha={"v3": sha})
    dve_ops._SUB_OPCODE_FOR_NAME[name] = row
    dve_ops.OPS.append(op)
    dve_ops.CUSTOM_DVE_SPECS[name] = op.spec
    return op


EXP2 = _make_exp2_op()

_nc_cache: dict = {}


def _build(repeat: int = 1):
    nc = bacc.Bacc("TRN2", target_bir_lowering=False)
    q_d = nc.dram_tensor("q", [HPC * CH, L], dt.float32, kind="ExternalInput")
    k_d = nc.dram_tensor("k", [HPC * CH, L], dt.float32, kind="ExternalInput")
    v_d = nc.dram_tensor("v", [HPC * CH, L], dt.float32, kind="ExternalInput")
    o_d = nc.dram_tensor("o", [HPC * CH, L], dt.float32, kind="ExternalOutput")

    windows = []
    j = 0
    while j < NJ:
        windows.append((j, min(WSZ, NJ - j)))
        j += WSZ

    # Chain every TensorE instruction in emission order (sync=False =
    # order-only, no semaphore); see module docstring.
    prev_pe = [None]

    def chain_pe(bi):
        if prev_pe[0] is not None:
            add_dep_helper(bi.ins, prev_pe[0].ins, sync=False, reason="pe order")
        prev_pe[0] = bi
        return bi

    with tile.TileContext(nc) as tc:
        with (
            tc.tile_pool(name="singles", bufs=1) as singles,
            tc.tile_pool(name="expw", bufs=5) as expp,
            tc.tile_pool(name="outp", bufs=3) as outp,
            tc.tile_pool(name="tails", bufs=2) as tails,
            tc.tile_pool(name="stp", bufs=3, space="PSUM") as stp,
            tc.tile_pool(name="pvp", bufs=2, space="PSUM") as pvp,
        ):
            # DVE exp constants: per-partition exponent-bias column
            offt = singles.tile([128, 1], dt.float32)
            nc.vector.memset(offt[:], OFF)
            # ScalarE exp bias column (ln2/2)
            actb = singles.tile([128, 1], dt.float32)
            nc.vector.memset(actb[:], ACT_BIAS)

            # K and Q for both heads packed on the partition axis
            # (head h occupies partitions 64h..64h+63), cast fp32->fp16
            # with the QK_PRE prescale folded in.
            k_r = singles.tile([128, L], dt.float16)
            q_r = singles.tile([128, L], dt.float16)
            # Starter tiles: the first windows need only q[:, 0:512]
            # (t-chunk 0) and k[:, 0:1024] (s-tiles 0-7).
            qs0 = singles.tile([128, 512], dt.float32)
            ks0 = singles.tile([128, 1024], dt.float32)
            ks1 = singles.tile([128, 1024], dt.float32)
            nc.sync.dma_start(qs0[:], q_d[:, 0:512])
            nc.sync.dma_start(ks0[:], k_d[:, 0:1024])
            nc.sync.dma_start(ks1[:], k_d[:, 1024:2048])
            nc.vector.tensor_scalar_mul(q_r[:, 0:512], qs0[:], QK_PRE)
            nc.vector.tensor_scalar_mul(k_r[:, 0:1024], ks0[:], QK_PRE)
            nc.vector.tensor_scalar_mul(k_r[:, 1024:2048], ks1[:], QK_PRE)

            # V (for the PE transposes, needed from ~10us) loads on the
            # sync queue ahead of the bulk q/k halves
            v_both = singles.tile([128, L], dt.float32)
            nc.sync.dma_start(v_both[:, 0:2048], v_d[:, 0:2048])
            nc.sync.dma_start(v_both[:, 2048:L], v_d[:, 2048:L])

            half = L // 2
            raws = {}
            for part in range(2):
                csl = slice(half * part, half * (part + 1))
                eng = nc.sync if part == 0 else nc.gpsimd
                for nm, src in (("q", q_d), ("k", k_d)):
                    if nm == "k" and part == 0:
                        continue  # fully covered by the k starter tiles
                    raw = singles.tile([128, half], dt.float32, name=f"{nm}raw{part}")
                    raws[(nm, part)] = raw
                    eng.dma_start(raw[:], src[:, csl])
            # bulk cast pieces, emitted lazily inside chunk 0's windows
            cast_queue = []
            for nm, r_dst, lo, hi in (
                ("k", k_r, 2048, 3072),
                ("k", k_r, 3072, 4096),
                ("q", q_r, 512, 1536),
                ("q", q_r, 1536, 2048),
                ("q", q_r, 2048, 3072),
                ("q", q_r, 3072, 4096),
            ):
                pieces = []
                for s0 in range(lo, hi, 1024):
                    s1 = min(s0 + 1024, hi)
                    part = s0 // half
                    pieces.append(
                        (r_dst, s0, s1, raws[(nm, part)], s0 - half * part)
                    )
                cast_queue.extend(pieces)

            def emit_casts(n):
                for _ in range(min(n, len(cast_queue))):
                    r_dst, s0, s1, raw, r0 = cast_queue.pop(0)
                    nc.vector.tensor_scalar_mul(
                        r_dst[:, s0:s1], raw[:, r0 : r0 + (s1 - s0)], QK_PRE
                    )

            # W_h[:, j, :] = [V^T ; ones] s-tile j: [128 s, 65] fp16, built
            # via xbar DMA transpose (t[p, j, c] = v16[c, 128j+p]) in 4
            # pieces of 8 j-tiles so the first PVs aren't blocked on the
            # full transpose.
            ws = []
            for h in range(HPC):
                w_h = singles.tile([128, NJ, 65], dt.float16, tag=f"W{h}")
                nc.vector.memset(w_h[:, :, 64:65], 1.0)
                ws.append(w_h)
            v16 = singles.tile([128, L], dt.float16)
            NPIECE = 4
            PW = L // NPIECE  # 1024 cols -> 8 j-tiles per piece
            tcur = [0]
            PE_TRANSPOSE = bool(os.environ.get("KERNEL_PE_TRANSPOSE"))
            if PE_TRANSPOSE:
                ident = singles.tile([128, 128], dt.float32)
                make_identity(nc, ident[:])

            def emit_transposes():
                p = tcur[0]
                if p >= NPIECE:
                    return
                tcur[0] += 1
                if PE_TRANSPOSE:
                    for h in range(HPC):
                        for j in range(8 * p, 8 * (p + 1)):
                            pt = stp.tile([128, 64], dt.float32, tag="st", name="pt")
                            chain_pe(
                                nc.tensor.transpose(
                                    pt[:],
                                    v_both[
                                        64 * h : 64 * h + 64, 128 * j : 128 * (j + 1)
                                    ],
                                    ident[64 * h : 64 * h + 64, 64 * h : 64 * h + 64],
                                )
                            )
                            nc.vector.tensor_copy(ws[h][:, j, 0:64], pt[:])
                    return
                csl = slice(PW * p, PW * (p + 1))
                nc.vector.tensor_copy(v16[:, csl], v_both[:, csl])
                jsl = slice(8 * p, 8 * (p + 1))
                for h in range(HPC):
                    nc.scalar.dma_start_transpose(
                        ws[h][:, jsl, 0:64], v16[64 * h : 64 * h + 64, csl]
                    )

            def emit_tail(i, h, pv_ps):
                tsl = slice(NT * i, NT * (i + 1))
                # copy PSUM->SBUF first so the pv bank frees immediately
                pv_sb = tails.tile([65, NT], dt.float32, tag="pvsb")
                nc.vector.tensor_copy(pv_sb[:], pv_ps[:])
                den = tails.tile([1, NT], dt.float32, tag="den")
                nc.vector.tensor_copy(den[:], pv_sb[64:65, :])
                d_bc = tails.tile([64, NT], dt.float32, tag="dbc")
                nc.gpsimd.partition_broadcast(d_bc[:], den[:])
                r_bc = tails.tile([64, NT], dt.float32, tag="rbc")
                if os.environ.get("KERNEL_EXACT_RECIP"):
                    nc.vector.reciprocal(r_bc[:], d_bc[:])
                else:
                    nc.vector.reciprocal_approx_fast(out=r_bc[:], in_=d_bc[:])
                o_sb = outp.tile([64, NT], dt.float32, tag="o")
                nc.vector.tensor_mul(o_sb[:], pv_sb[0:64, :], r_bc[:])
                nc.sync.dma_start(o_d[64 * h : 64 * h + 64, tsl], o_sb[:])

            def flush(pend):
                i, j0, cnt, ews, pvs = pend
                for h in range(HPC):
                    _pv(nc, pvs[h], ws[h], (ews[h], j0, cnt), chain_pe)
                if j0 + cnt == NJ:
                    for h in range(HPC):
                        emit_tail(i, h, pvs[h])

            # Global software pipeline over (t-chunk, window): QK(g) and
            # exp(g) are emitted one window ahead of PV(g-1).
            pend = None
            gwin = [0]
            gexp = [0]  # (window, head) exp counter for engine assignment
            for _rep in range(repeat):
                for i in range(TCH):
                    tsl = slice(NT * i, NT * (i + 1))
                    pvs = [
                        pvp.tile([65, NT], dt.float32, tag="pv", name=f"pv{h}")
                        for h in range(HPC)
                    ]
                    for j0, cnt in windows:
                        if cast_queue and gwin[0] in (2, 5, 8, 10, 11, 12, 13):
                            emit_casts(1)
                        gwin[0] += 1
                        sts = [
                            stp.tile(
                                [128, WSZ * NT], dt.float32, tag="st", name=f"st{h}"
                            )
                            for h in range(HPC)
                        ]
                        # interleave heads so row-packed QK pairs overlap on PE
                        for jj in range(cnt):
                            j = j0 + jj
                            for h in range(HPC):
                                hsl = slice(64 * h, 64 * h + 64)
                                chain_pe(
                                    nc.tensor.matmul(
                                        sts[h][:, NT * jj : NT * (jj + 1)],
                                        k_r[hsl, 128 * j : 128 * (j + 1)],
                                        q_r[hsl, tsl],
                                        start=True,
                                        stop=True,
                                        tile_position=(64 * h, 0),
                                    )
                                )
                        if tcur[0] < NPIECE:
                            emit_transposes()
                        ews = []
                        for h in range(HPC):
                            ew = expp.tile([128, WSZ * NT], dt.float16, tag="ew")
                            g = gexp[0]
                            gexp[0] += 1
                            use_dve = (g * DVE_NUM) % DVE_DEN < DVE_NUM
                            if use_dve:
                                nc.vector._custom_dve(
                                    EXP2,
                                    out=ew[:, 0 : cnt * NT].bitcast(dt.int16),
                                    in0=sts[h][:, 0 : cnt * NT],
                                    in1=offt[:].to_broadcast((128, cnt * NT)),
                                    s0=M_MAGIC,
                                    s1=G2,
                                    imm2=G1,
                                )
                            else:
                                nc.scalar.activation(
                                    ew[:, 0 : cnt * NT],
                                    sts[h][:, 0 : cnt * NT],
                                    mybir.ActivationFunctionType.Exp,
                                    bias=actb[:],
                                    scale=ACT_SCALE,
                                )
                            ews.append(ew)
                        if pend is not None:
                            flush(pend)
                        pend = (i, j0, cnt, ews, pvs)
            flush(pend)

    nc.compile()
    return nc


PV_CHAIN = not os.environ.get("KERNEL_PV_UNCHAIN")


def _pv(nc, pv_ps, w_h, pending, chain_pe):
    # Column-split PV: j=0 runs as one M=65 matmul with start=True (sets
    # has_written for the whole tile); later s-tiles split into an M=64
    # channel matmul (col groups 0-1) and an M=1 denominator matmul (col
    # group 2) with disjoint output partitions in the same bank. The two
    # run concurrently on the array and each pair's LDWEIGHTS overlaps
    # the other's stream, so a pair costs ~N cycles instead of the
    # serial LDW+MM of a single M=65 matmul.
    ew, j0, cnt = pending
    for jj in range(cnt):
        j = j0 + jj
        esl = slice(NT * jj, NT * (jj + 1))
        if j == 0:
            bi = nc.tensor.matmul(
                pv_ps[:],
                w_h[:, j, :],
                ew[:, esl],
                start=True,
                stop=False,
                tile_position=(0, 0),
            )
            if PV_CHAIN:
                chain_pe(bi)
            continue
        last = j == NJ - 1
        for c0, cw, pos in ((0, 64, (0, 0)), (64, 1, (0, 64))):
            bi = nc.tensor.matmul(
                pv_ps[c0 : c0 + cw, :],
                w_h[:, j, c0 : c0 + cw],
                ew[:, esl],
                start=False,
                stop=last and (c0 == 64),
                tile_position=pos,
            )
            if PV_CHAIN:
                chain_pe(bi)


def _get_nc(repeat: int = 1):
    if repeat not in _nc_cache:
        _nc_cache[repeat] = _build(repeat)
    return _nc_cache[repeat]


def kernel(qkv: np.ndarray, _repeat: int = 1) -> np.ndarray:
    qkv = np.asarray(qkv)
    bs, width, length = qkv.shape
    assert (bs, width, length) == (BS, 3 * NUM_HEADS * CH, L), qkv.shape
    hw = NUM_HEADS * CH

    nc = _get_nc(_repeat)
    in_maps = []
    for c in range(NCORES):
        qs, ks, vs = [], [], []
        for i in range(HPC):
            bh = c * HPC + i
            b, h = bh // NUM_HEADS, bh % NUM_HEADS
            qs.append(qkv[b, h * CH : (h + 1) * CH, :])
            ks.append(qkv[b, hw + h * CH : hw + (h + 1) * CH, :])
            vs.append(qkv[b, 2 * hw + h * CH : 2 * hw + (h + 1) * CH, :])
        in_maps.append(
            {
                "q": np.ascontiguousarray(np.concatenate(qs, axis=0)),
                "k": np.ascontiguousarray(np.concatenate(ks, axis=0)),
                "v": np.ascontiguousarray(np.concatenate(vs, axis=0)),
            }
        )

    res = run_bass_kernel_spmd(nc, in_maps, list(range(NCORES)))

    out = np.empty((BS, hw, L), np.float32)
    for c in range(NCORES):
        oc = res.results[c]["o"]
        for i in range(HPC):
            bh = c * HPC + i
            b, h = bh // NUM_HEADS, bh % NUM_HEADS
            out[b, h * CH : (h + 1) * CH, :] = oc[i * CH : (i + 1) * CH]
    return out


# revision 31
# speedup vs baseline: 1.2583x; 1.2583x over previous
"""Multi-head attention (bs=2, heads=8, ch=64, len=4096) on 8 Trainium2 cores.

Sharding: bs*heads = 16 head-problems, 2 per core (head/data parallel,
no cross-core communication).

Per-core algorithm (per head, seq len L=4096, ch=64):
  - S^T tiles: for s-tile j (128 rows) and t-chunk i (512 cols):
        st[s,t] = sum_c K[c,s] Q[c,t]         (PE, fp16, K=64)
    Head 0 uses array rows 0-63, head 1 rows 64-127 (tile_position),
    so adjacent A/B QK matmuls overlap in the array (~2x).
    q,k are prescaled by sqrt(1024*log2(e)*ch^-0.5) at cast time, so the
    PSUM logits arrive as Z = 1024*log2(e)*scale*logit.
  - expS^T: softmax numerator tiles in fp16, computed on TWO engines to
    split the 33.5M-exp/core wall (ScalarE alone = 1 elem/lane/cycle
    @1.2GHz = ~300us):
      * ScalarE windows: ACT exp with scale=ln2/1024, bias=ln2/2
        -> sqrt(2)*exp(scale*logit) in fp16.
      * DVE windows: custom 8-slice DVE op EXP2_FP16_ANT writing int16
        fp16-bits directly: magic-add floor split z=e+f, quadratic
        2^f-1 mantissa polynomial, +exponent bias, RNE int16 convert at
        the write port; bitcast fp16 = sqrt(2)*2^z (max rel err ~3e-3).
    Both paths carry the same global sqrt(2) factor, which cancels in
    the softmax normalize.
  - PV with folded denominator: lhsT = [V^T ; ones] ([128, 65] fp16,
    built once per head via PE transpose), accumulated over 32 s-tiles:
        pv[c,t]  = sum_s V[c,s] expS^T[s,t]   (c = 0..63)
        pv[64,t] = sum_s expS^T[s,t]          (softmax denominator)
  - normalize: o[c,t] = pv[c,t] * (1/pv[64,t])
    (custom-DVE fast reciprocal ~51ULP, GpSimd partition-broadcast,
    DVE multiply)

Structural notes (from NTFF profiling):
  - every TensorE instruction is chained in emission order
    (add_dep_helper sync=False): the Tile scheduler otherwise groups
    same-row-group K=64 matmuls, which serialize at 427ns (LDWEIGHTS
    cannot be pulled ahead); the emitted order runs warm with A/B QK
    pairs overlapping.
  - exp windows are 2 PSUM banks x 3 bufs so QK runs ahead of the two
    exp engines.
  - input DMAs are [128]-partition transfers with small starter tiles
    so compute begins at ~10us while the bulk streams.
"""

import os
import sys

sys.path.insert(0, "/opt/trn_rl_repo")

import math

import numpy as np
from concourse import mybir, tile, bacc, dve_ops
from concourse import bass_utils as _bu
from concourse.bass_utils import run_bass_kernel_spmd

if os.environ.get("KERNEL_LDW_OPT"):
    _orig_run_command = _bu.run_command

    def _run_command_ldw(cmd, *a, **k):
        cmd = [
            "--enable-ldw-opt=true" if c == "--enable-ldw-opt=false" else c
            for c in cmd
        ]
        return _orig_run_command(cmd, *a, **k)

    _bu.run_command = _run_command_ldw
from concourse.dve_spec import Spec, Src0, Src1, C0, C1, C2, lower
from concourse.dve_uop import DveOpSpec
from concourse.masks import make_identity
from concourse.tile_rust import add_dep_helper

dt = mybir.dt

NUM_HEADS = 8
BS = 2
CH = 64
L = 4096
NCORES = 8
HPC = BS * NUM_HEADS // NCORES  # heads per core = 2
NT = 512  # t-chunk (matmul moving dim / PSUM bank)
TCH = L // NT  # 8 t-chunks
NJ = L // 128  # 32 s-tiles
WSZ = 2  # s-tiles per exp window (2 PSUM banks; bufs=3 -> 6 banks)
SCALE = float(CH) ** -0.5

# --- exp split constants ----------------------------------------------------
LOG2E = math.log2(math.e)
QK_PRE = math.sqrt(1024.0 * LOG2E * SCALE)  # q,k cast-time prescale
ACT_SCALE = math.log(2.0) / 1024.0  # ScalarE: exp(Z*ln2/1024 + ln2/2)
ACT_BIAS = 0.5 * math.log(2.0)
# quadratic minimax fit of 2^g-1 on g=[0,1] (bitcast rel err <= 2.8e-3)
C0_POLY = 0.41470000
C1_POLY = 0.99176072
C2_POLY = 0.33573335
M_MAGIC = float(1.5 * 2**33)
G2 = C2_POLY / 1024.0
G1 = C1_POLY
OFF = 15360.0 + 1024.0 * C0_POLY
# of each 16 consecutive (window, head) exp tiles, this many go to the DVE
DVE_NUM = int(os.environ.get("KERNEL_DVE_NUM", "7"))
DVE_DEN = int(os.environ.get("KERNEL_DVE_DEN", "16"))

# --- custom DVE op: int16 fp16-bits of sqrt(2)*2^(Z/1024) -------------------
_t = Src0 + C0
_E = _t - C0
_F = Src0 - _E
_u = (_F * C1 + C2) * _F
_EXP2_BODY = (_E + _u) + Src1


def _exp2_ref(in0, in1, s0, s1, imm2):
    z = in0.astype(np.float32)
    tt = (z + np.float32(s0)).astype(np.float32)
    Ee = (tt - np.float32(s0)).astype(np.float32)
    Ff = (z - Ee).astype(np.float32)
    uu = ((Ff * np.float32(s1) + np.float32(imm2)) * Ff).astype(np.float32)
    return (Ee + uu) + in1


def _make_exp2_op():
    name = "EXP2_FP16_ANT"
    for op in dve_ops.OPS:
        if op.name == name:
            return op
    spec = Spec(body=_EXP2_BODY, reference=_exp2_ref)
    row = dve_ops._CUSTOM_DVE_ROW_BASE + len(dve_ops.OPS)
    sha = DveOpSpec(
        name=name, opcode=row, uops=lower(spec, ver="v3"), rd1_en=True
    ).sha("v3")
    op = dve_ops.DveOp(name, spec, subdim=False, uops_sha={"v3": sha})
    dve_ops._SUB_OPCODE_FOR_NAME[name] = row
    dve_ops.OPS.append(op)
    dve_ops.CUSTOM_DVE_SPECS[name] = op.spec
    return op


EXP2 = _make_exp2_op()

_nc_cache: dict = {}


def _build(repeat: int = 1):
    nc = bacc.Bacc("TRN2", target_bir_lowering=False)
    q_d = nc.dram_tensor("q", [HPC * CH, L], dt.float32, kind="ExternalInput")
    k_d = nc.dram_tensor("k", [HPC * CH, L], dt.float32, kind="ExternalInput")
    v_d = nc.dram_tensor("v", [HPC * CH, L], dt.float32, kind="ExternalInput")
    o_d = nc.dram_tensor("o", [HPC * CH, L], dt.float32, kind="ExternalOutput")

    windows = []
    j = 0
    while j < NJ:
        windows.append((j, min(WSZ, NJ - j)))
        j += WSZ

    # Chain every TensorE instruction in emission order (sync=False =
    # order-only, no semaphore); see module docstring.
    prev_pe = [None]

    def chain_pe(bi):
        if prev_pe[0] is not None:
            add_dep_helper(bi.ins, prev_pe[0].ins, sync=False, reason="pe order")
        prev_pe[0] = bi
        return bi

    with tile.TileContext(nc) as tc:
        with (
            tc.tile_pool(name="singles", bufs=1) as singles,
            tc.tile_pool(name="expw", bufs=5) as expp,
            tc.tile_pool(name="outp", bufs=3) as outp,
            tc.tile_pool(name="tails", bufs=2) as tails,
            tc.tile_pool(name="stp", bufs=3, space="PSUM") as stp,
            tc.tile_pool(name="pvp", bufs=2, space="PSUM") as pvp,
        ):
            # DVE exp constants: per-partition exponent-bias column
            offt = singles.tile([128, 1], dt.float32)
            nc.vector.memset(offt[:], OFF)
            # ScalarE exp bias column (ln2/2)
            actb = singles.tile([128, 1], dt.float32)
            nc.vector.memset(actb[:], ACT_BIAS)

            # K and Q for both heads packed on the partition axis
            # (head h occupies partitions 64h..64h+63), cast fp32->fp16
            # with the QK_PRE prescale folded in.
            k_r = singles.tile([128, L], dt.float16)
            q_r = singles.tile([128, L], dt.float16)
            # Starter tiles: the first windows need only q[:, 0:512]
            # (t-chunk 0) and k[:, 0:1024] (s-tiles 0-7).
            qs0 = singles.tile([128, 512], dt.float32)
            ks0 = singles.tile([128, 1024], dt.float32)
            ks1 = singles.tile([128, 1024], dt.float32)
            nc.sync.dma_start(qs0[:], q_d[:, 0:512])
            nc.sync.dma_start(ks0[:], k_d[:, 0:1024])
            nc.sync.dma_start(ks1[:], k_d[:, 1024:2048])
            nc.vector.tensor_scalar_mul(q_r[:, 0:512], qs0[:], QK_PRE)
            nc.vector.tensor_scalar_mul(k_r[:, 0:1024], ks0[:], QK_PRE)
            nc.vector.tensor_scalar_mul(k_r[:, 1024:2048], ks1[:], QK_PRE)

            # V (for the PE transposes, needed from ~10us) loads on the
            # sync queue ahead of the bulk q/k halves
            v_both = singles.tile([128, L], dt.float32)
            nc.sync.dma_start(v_both[:, 0:2048], v_d[:, 0:2048])
            nc.sync.dma_start(v_both[:, 2048:L], v_d[:, 2048:L])

            half = L // 2
            raws = {}
            for part in range(2):
                csl = slice(half * part, half * (part + 1))
                eng = nc.sync if part == 0 else nc.gpsimd
                for nm, src in (("q", q_d), ("k", k_d)):
                    if nm == "k" and part == 0:
                        continue  # fully covered by the k starter tiles
                    raw = singles.tile([128, half], dt.float32, name=f"{nm}raw{part}")
                    raws[(nm, part)] = raw
                    eng.dma_start(raw[:], src[:, csl])
            # bulk cast pieces, emitted lazily inside chunk 0's windows
            cast_queue = []
            for nm, r_dst, lo, hi in (
                ("k", k_r, 2048, 3072),
                ("k", k_r, 3072, 4096),
                ("q", q_r, 512, 1536),
                ("q", q_r, 1536, 2048),
                ("q", q_r, 2048, 3072),
                ("q", q_r, 3072, 4096),
            ):
                pieces = []
                for s0 in range(lo, hi, 1024):
                    s1 = min(s0 + 1024, hi)
                    part = s0 // half
                    pieces.append(
                        (r_dst, s0, s1, raws[(nm, part)], s0 - half * part)
                    )
                cast_queue.extend(pieces)

            def emit_casts(n):
                for _ in range(min(n, len(cast_queue))):
                    r_dst, s0, s1, raw, r0 = cast_queue.pop(0)
                    nc.vector.tensor_scalar_mul(
                        r_dst[:, s0:s1], raw[:, r0 : r0 + (s1 - s0)], QK_PRE
                    )

            # W_h[:, j, :] = [V^T ; ones] s-tile j: [128 s, 65] fp16, built
            # via xbar DMA transpose (t[p, j, c] = v16[c, 128j+p]) in 4
            # pieces of 8 j-tiles so the first PVs aren't blocked on the
            # full transpose.
            ws = []
            for h in range(HPC):
                w_h = singles.tile([128, NJ, 65], dt.float16, tag=f"W{h}")
                nc.vector.memset(w_h[:, :, 64:65], 1.0)
                ws.append(w_h)
            v16 = singles.tile([128, L], dt.float16)
            NPIECE = 4
            PW = L // NPIECE  # 1024 cols -> 8 j-tiles per piece
            tcur = [0]
            PE_TRANSPOSE = bool(os.environ.get("KERNEL_PE_TRANSPOSE"))
            if PE_TRANSPOSE:
                ident = singles.tile([128, 128], dt.float32)
                make_identity(nc, ident[:])

            def emit_transposes():
                p = tcur[0]
                if p >= NPIECE:
                    return
                tcur[0] += 1
                if PE_TRANSPOSE:
                    for h in range(HPC):
                        for j in range(8 * p, 8 * (p + 1)):
                            pt = stp.tile([128, 64], dt.float32, tag="st", name="pt")
                            chain_pe(
                                nc.tensor.transpose(
                                    pt[:],
                                    v_both[
                                        64 * h : 64 * h + 64, 128 * j : 128 * (j + 1)
                                    ],
                                    ident[64 * h : 64 * h + 64, 64 * h : 64 * h + 64],
                                )
                            )
                            nc.vector.tensor_copy(ws[h][:, j, 0:64], pt[:])
                    return
                csl = slice(PW * p, PW * (p + 1))
                nc.vector.tensor_copy(v16[:, csl], v_both[:, csl])
                jsl = slice(8 * p, 8 * (p + 1))
                for h in range(HPC):
                    nc.scalar.dma_start_transpose(
                        ws[h][:, jsl, 0:64], v16[64 * h : 64 * h + 64, csl]
                    )

            def emit_tail(i, h, pv_ps):
                tsl = slice(NT * i, NT * (i + 1))
                # copy PSUM->SBUF first so the pv bank frees immediately
                pv_sb = tails.tile([65, NT], dt.float32, tag="pvsb")
                nc.vector.tensor_copy(pv_sb[:], pv_ps[:])
                den = tails.tile([1, NT], dt.float32, tag="den")
                nc.vector.tensor_copy(den[:], pv_sb[64:65, :])
                d_bc = tails.tile([64, NT], dt.float32, tag="dbc")
                nc.gpsimd.partition_broadcast(d_bc[:], den[:])
                r_bc = tails.tile([64, NT], dt.float32, tag="rbc")
                if os.environ.get("KERNEL_EXACT_RECIP"):
                    nc.vector.reciprocal(r_bc[:], d_bc[:])
                else:
                    nc.vector.reciprocal_approx_fast(out=r_bc[:], in_=d_bc[:])
                o_sb = outp.tile([64, NT], dt.float32, tag="o")
                nc.vector.tensor_mul(o_sb[:], pv_sb[0:64, :], r_bc[:])
                nc.sync.dma_start(o_d[64 * h : 64 * h + 64, tsl], o_sb[:])

            def flush(pend):
                # interleave heads so consecutive PV matmuls alternate PSUM
                # banks (same-bank back-to-back matmuls serialize fully)
                i, j0, cnt, ews, pvs = pend
                for jj in range(cnt):
                    j = j0 + jj
                    esl = slice(NT * jj, NT * (jj + 1))
                    for h in range(HPC):
                        chain_pe(
                            nc.tensor.matmul(
                                pvs[h][:],
                                ws[h][:, j, :],
                                ews[h][:, esl],
                                start=(j == 0),
                                stop=(j == NJ - 1),
                            )
                        )
                if j0 + cnt == NJ:
                    for h in range(HPC):
                        emit_tail(i, h, pvs[h])

            # Global software pipeline over (t-chunk, window): QK(g) and
            # exp(g) are emitted one window ahead of PV(g-1).
            pend = None
            gwin = [0]
            gexp = [0]  # (window, head) exp counter for engine assignment
            for _rep in range(repeat):
                for i in range(TCH):
                    tsl = slice(NT * i, NT * (i + 1))
                    pvs = [
                        pvp.tile([65, NT], dt.float32, tag="pv", name=f"pv{h}")
                        for h in range(HPC)
                    ]
                    for j0, cnt in windows:
                        if cast_queue and gwin[0] in (2, 5, 8, 10, 11, 12, 13):
                            emit_casts(1)
                        gwin[0] += 1
                        sts = [
                            stp.tile(
                                [128, WSZ * NT], dt.float32, tag="st", name=f"st{h}"
                            )
                            for h in range(HPC)
                        ]
                        # interleave heads so row-packed QK pairs overlap on PE
                        for jj in range(cnt):
                            j = j0 + jj
                            for h in range(HPC):
                                hsl = slice(64 * h, 64 * h + 64)
                                chain_pe(
                                    nc.tensor.matmul(
                                        sts[h][:, NT * jj : NT * (jj + 1)],
                                        k_r[hsl, 128 * j : 128 * (j + 1)],
                                        q_r[hsl, tsl],
                                        start=True,
                                        stop=True,
                                        tile_position=(64 * h, 0),
                                    )
                                )
                        if tcur[0] < NPIECE:
                            emit_transposes()
                        ews = []
                        for h in range(HPC):
                            ew = expp.tile([128, WSZ * NT], dt.float16, tag="ew")
                            g = gexp[0]
                            gexp[0] += 1
                            use_dve = (g * DVE_NUM) % DVE_DEN < DVE_NUM
                            if use_dve:
                                nc.vector._custom_dve(
                                    EXP2,
                                    out=ew[:, 0 : cnt * NT].bitcast(dt.int16),
                                    in0=sts[h][:, 0 : cnt * NT],
                                    in1=offt[:].to_broadcast((128, cnt * NT)),
                                    s0=M_MAGIC,
                                    s1=G2,
                                    imm2=G1,
                                )
                            else:
                                nc.scalar.activation(
                                    ew[:, 0 : cnt * NT],
                                    sts[h][:, 0 : cnt * NT],
                                    mybir.ActivationFunctionType.Exp,
                                    bias=actb[:],
                                    scale=ACT_SCALE,
                                )
                            ews.append(ew)
                        if pend is not None:
                            flush(pend)
                        pend = (i, j0, cnt, ews, pvs)
            flush(pend)

    nc.compile()
    return nc


PV_CHAIN = not os.environ.get("KERNEL_PV_UNCHAIN")


def _pv(nc, pv_ps, w_h, pending, chain_pe):
    # Column-split PV: j=0 runs as one M=65 matmul with start=True (sets
    # has_written for the whole tile); later s-tiles split into an M=64
    # channel matmul (col groups 0-1) and an M=1 denominator matmul (col
    # group 2) with disjoint output partitions in the same bank. The two
    # run concurrently on the array and each pair's LDWEIGHTS overlaps
    # the other's stream, so a pair costs ~N cycles instead of the
    # serial LDW+MM of a single M=65 matmul.
    ew, j0, cnt = pending
    for jj in range(cnt):
        j = j0 + jj
        esl = slice(NT * jj, NT * (jj + 1))
        if j == 0:
            bi = nc.tensor.matmul(
                pv_ps[:],
                w_h[:, j, :],
                ew[:, esl],
                start=True,
                stop=False,
                tile_position=(0, 0),
            )
            if PV_CHAIN:
                chain_pe(bi)
            continue
        last = j == NJ - 1
        for c0, cw, pos in ((0, 64, (0, 0)), (64, 1, (0, 64))):
            bi = nc.tensor.matmul(
                pv_ps[c0 : c0 + cw, :],
                w_h[:, j, c0 : c0 + cw],
                ew[:, esl],
                start=False,
                stop=last and (c0 == 64),
                tile_position=pos,
            )
            if PV_CHAIN:
                chain_pe(bi)


def _get_nc(repeat: int = 1):
    if repeat not in _nc_cache:
        _nc_cache[repeat] = _build(repeat)
    return _nc_cache[repeat]


def kernel(qkv: np.ndarray, _repeat: int = 1) -> np.ndarray:
    qkv = np.asarray(qkv)
    bs, width, length = qkv.shape
    assert (bs, width, length) == (BS, 3 * NUM_HEADS * CH, L), qkv.shape
    hw = NUM_HEADS * CH

    nc = _get_nc(_repeat)
    in_maps = []
    for c in range(NCORES):
        qs, ks, vs = [], [], []
        for i in range(HPC):
            bh = c * HPC + i
            b, h = bh // NUM_HEADS, bh % NUM_HEADS
            qs.append(qkv[b, h * CH : (h + 1) * CH, :])
            ks.append(qkv[b, hw + h * CH : hw + (h + 1) * CH, :])
            vs.append(qkv[b, 2 * hw + h * CH : 2 * hw + (h + 1) * CH, :])
        in_maps.append(
            {
                "q": np.ascontiguousarray(np.concatenate(qs, axis=0)),
                "k": np.ascontiguousarray(np.concatenate(ks, axis=0)),
                "v": np.ascontiguousarray(np.concatenate(vs, axis=0)),
            }
        )

    res = run_bass_kernel_spmd(nc, in_maps, list(range(NCORES)))

    out = np.empty((BS, hw, L), np.float32)
    for c in range(NCORES):
        oc = res.results[c]["o"]
        for i in range(HPC):
            bh = c * HPC + i
            b, h = bh // NUM_HEADS, bh % NUM_HEADS
            out[b, h * CH : (h + 1) * CH, :] = oc[i * CH : (i + 1) * CH]
    return out


# revision 32
# speedup vs baseline: 1.2883x; 1.0238x over previous
"""Multi-head attention (bs=2, heads=8, ch=64, len=4096) on 8 Trainium2 cores.

Sharding: bs*heads = 16 head-problems, 2 per core (head/data parallel,
no cross-core communication).

Per-core algorithm (per head, seq len L=4096, ch=64):
  - S^T tiles: for s-tile j (128 rows) and t-chunk i (512 cols):
        st[s,t] = sum_c K[c,s] Q[c,t]         (PE, fp16, K=64)
    Head 0 uses array rows 0-63, head 1 rows 64-127 (tile_position),
    so adjacent A/B QK matmuls overlap in the array (~2x).
    q,k are prescaled by sqrt(1024*log2(e)*ch^-0.5) at cast time, so the
    PSUM logits arrive as Z = 1024*log2(e)*scale*logit.
  - expS^T: softmax numerator tiles in fp16, computed on TWO engines to
    split the 33.5M-exp/core wall (ScalarE alone = 1 elem/lane/cycle
    @1.2GHz = ~300us):
      * ScalarE windows: ACT exp with scale=ln2/1024, bias=ln2/2
        -> sqrt(2)*exp(scale*logit) in fp16.
      * DVE windows: custom 8-slice DVE op EXP2_FP16_ANT writing int16
        fp16-bits directly: magic-add floor split z=e+f, quadratic
        2^f-1 mantissa polynomial, +exponent bias, RNE int16 convert at
        the write port; bitcast fp16 = sqrt(2)*2^z (max rel err ~3e-3).
    Both paths carry the same global sqrt(2) factor, which cancels in
    the softmax normalize.
  - PV with folded denominator: lhsT = [V^T ; ones] ([128, 65] fp16,
    built once per head via PE transpose), accumulated over 32 s-tiles:
        pv[c,t]  = sum_s V[c,s] expS^T[s,t]   (c = 0..63)
        pv[64,t] = sum_s expS^T[s,t]          (softmax denominator)
  - normalize: o[c,t] = pv[c,t] * (1/pv[64,t])
    (custom-DVE fast reciprocal ~51ULP, GpSimd partition-broadcast,
    DVE multiply)

Structural notes (from NTFF profiling):
  - every TensorE instruction is chained in emission order
    (add_dep_helper sync=False): the Tile scheduler otherwise groups
    same-row-group K=64 matmuls, which serialize at 427ns (LDWEIGHTS
    cannot be pulled ahead); the emitted order runs warm with A/B QK
    pairs overlapping.
  - exp windows are 2 PSUM banks x 3 bufs so QK runs ahead of the two
    exp engines.
  - input DMAs are [128]-partition transfers with small starter tiles
    so compute begins at ~10us while the bulk streams.
"""

import os
import sys

sys.path.insert(0, "/opt/trn_rl_repo")

import math

import numpy as np
from concourse import mybir, tile, bacc, dve_ops
from concourse import bass_utils as _bu
from concourse.bass_utils import run_bass_kernel_spmd

if os.environ.get("KERNEL_LDW_OPT"):
    _orig_run_command = _bu.run_command

    def _run_command_ldw(cmd, *a, **k):
        cmd = [
            "--enable-ldw-opt=true" if c == "--enable-ldw-opt=false" else c
            for c in cmd
        ]
        return _orig_run_command(cmd, *a, **k)

    _bu.run_command = _run_command_ldw
from concourse.dve_spec import Spec, Src0, Src1, C0, C1, C2, lower
from concourse.dve_uop import DveOpSpec
from concourse.masks import make_identity
from concourse.tile_rust import add_dep_helper

dt = mybir.dt

NUM_HEADS = 8
BS = 2
CH = 64
L = 4096
NCORES = 8
HPC = BS * NUM_HEADS // NCORES  # heads per core = 2
NT = 512  # t-chunk (matmul moving dim / PSUM bank)
TCH = L // NT  # 8 t-chunks
NJ = L // 128  # 32 s-tiles
WSZ = 2  # s-tiles per exp window (2 PSUM banks; bufs=3 -> 6 banks)
SCALE = float(CH) ** -0.5

# --- exp split constants ----------------------------------------------------
LOG2E = math.log2(math.e)
QK_PRE = math.sqrt(1024.0 * LOG2E * SCALE)  # q,k cast-time prescale
ACT_SCALE = math.log(2.0) / 1024.0  # ScalarE: exp(Z*ln2/1024 + ln2/2)
ACT_BIAS = 0.5 * math.log(2.0)
# quadratic minimax fit of 2^g-1 on g=[0,1] (bitcast rel err <= 2.8e-3)
C0_POLY = 0.41470000
C1_POLY = 0.99176072
C2_POLY = 0.33573335
M_MAGIC = float(1.5 * 2**33)
G2 = C2_POLY / 1024.0
G1 = C1_POLY
OFF = 15360.0 + 1024.0 * C0_POLY
# of each 16 consecutive (window, head) exp tiles, this many go to the DVE
DVE_NUM = int(os.environ.get("KERNEL_DVE_NUM", "7"))
DVE_DEN = int(os.environ.get("KERNEL_DVE_DEN", "16"))

# --- custom DVE op: int16 fp16-bits of sqrt(2)*2^(Z/1024) -------------------
_t = Src0 + C0
_E = _t - C0
_F = Src0 - _E
_u = (_F * C1 + C2) * _F
_EXP2_BODY = (_E + _u) + Src1


def _exp2_ref(in0, in1, s0, s1, imm2):
    z = in0.astype(np.float32)
    tt = (z + np.float32(s0)).astype(np.float32)
    Ee = (tt - np.float32(s0)).astype(np.float32)
    Ff = (z - Ee).astype(np.float32)
    uu = ((Ff * np.float32(s1) + np.float32(imm2)) * Ff).astype(np.float32)
    return (Ee + uu) + in1


def _make_exp2_op():
    name = "EXP2_FP16_ANT"
    for op in dve_ops.OPS:
        if op.name == name:
            return op
    spec = Spec(body=_EXP2_BODY, reference=_exp2_ref)
    row = dve_ops._CUSTOM_DVE_ROW_BASE + len(dve_ops.OPS)
    sha = DveOpSpec(
        name=name, opcode=row, uops=lower(spec, ver="v3"), rd1_en=True
    ).sha("v3")
    op = dve_ops.DveOp(name, spec, subdim=False, uops_sha={"v3": sha})
    dve_ops._SUB_OPCODE_FOR_NAME[name] = row
    dve_ops.OPS.append(op)
    dve_ops.CUSTOM_DVE_SPECS[name] = op.spec
    return op


EXP2 = _make_exp2_op()

_nc_cache: dict = {}


def _build(repeat: int = 1):
    nc = bacc.Bacc("TRN2", target_bir_lowering=False)
    q_d = nc.dram_tensor("q", [HPC * CH, L], dt.float32, kind="ExternalInput")
    k_d = nc.dram_tensor("k", [HPC * CH, L], dt.float32, kind="ExternalInput")
    v_d = nc.dram_tensor("v", [HPC * CH, L], dt.float32, kind="ExternalInput")
    o_d = nc.dram_tensor("o", [HPC * CH, L], dt.float32, kind="ExternalOutput")

    windows = []
    j = 0
    while j < NJ:
        windows.append((j, min(WSZ, NJ - j)))
        j += WSZ

    # Chain every TensorE instruction in emission order (sync=False =
    # order-only, no semaphore); see module docstring.
    prev_pe = [None]

    def chain_pe(bi):
        if prev_pe[0] is not None:
            add_dep_helper(bi.ins, prev_pe[0].ins, sync=False, reason="pe order")
        prev_pe[0] = bi
        return bi

    with tile.TileContext(nc) as tc:
        with (
            tc.tile_pool(name="singles", bufs=1) as singles,
            tc.tile_pool(name="expw", bufs=5) as expp,
            tc.tile_pool(name="outp", bufs=3) as outp,
            tc.tile_pool(name="tails", bufs=2) as tails,
            tc.tile_pool(name="stp", bufs=3, space="PSUM") as stp,
            tc.tile_pool(name="pvp", bufs=2, space="PSUM") as pvp,
        ):
            # DVE exp constants: per-partition exponent-bias column
            offt = singles.tile([128, 1], dt.float32)
            nc.vector.memset(offt[:], OFF)
            # ScalarE exp bias column (ln2/2)
            actb = singles.tile([128, 1], dt.float32)
            nc.vector.memset(actb[:], ACT_BIAS)

            # K and Q for both heads packed on the partition axis
            # (head h occupies partitions 64h..64h+63), cast fp32->fp16
            # with the QK_PRE prescale folded in.
            k_r = singles.tile([128, L], dt.float16)
            q_r = singles.tile([128, L], dt.float16)
            # Starter tiles: the first windows need only q[:, 0:512]
            # (t-chunk 0) and k[:, 0:1024] (s-tiles 0-7).
            qs0 = singles.tile([128, 512], dt.float32)
            ks0 = singles.tile([128, 1024], dt.float32)
            ks1 = singles.tile([128, 1024], dt.float32)
            nc.sync.dma_start(qs0[:], q_d[:, 0:512])
            nc.sync.dma_start(ks0[:], k_d[:, 0:1024])
            nc.sync.dma_start(ks1[:], k_d[:, 1024:2048])
            nc.vector.tensor_scalar_mul(q_r[:, 0:512], qs0[:], QK_PRE)
            nc.vector.tensor_scalar_mul(k_r[:, 0:1024], ks0[:], QK_PRE)
            nc.vector.tensor_scalar_mul(k_r[:, 1024:2048], ks1[:], QK_PRE)

            # V (for the PE transposes, needed from ~10us) loads on the
            # sync queue ahead of the bulk q/k halves
            v_both = singles.tile([128, L], dt.float32)
            nc.sync.dma_start(v_both[:, 0:2048], v_d[:, 0:2048])
            nc.sync.dma_start(v_both[:, 2048:L], v_d[:, 2048:L])

            half = L // 2
            raws = {}
            for part in range(2):
                csl = slice(half * part, half * (part + 1))
                eng = nc.sync if part == 0 else nc.gpsimd
                for nm, src in (("q", q_d), ("k", k_d)):
                    if nm == "k" and part == 0:
                        continue  # fully covered by the k starter tiles
                    raw = singles.tile([128, half], dt.float32, name=f"{nm}raw{part}")
                    raws[(nm, part)] = raw
                    eng.dma_start(raw[:], src[:, csl])
            # bulk cast pieces, emitted lazily inside chunk 0's windows
            cast_queue = []
            for nm, r_dst, lo, hi in (
                ("k", k_r, 2048, 3072),
                ("k", k_r, 3072, 4096),
                ("q", q_r, 512, 1536),
                ("q", q_r, 1536, 2048),
                ("q", q_r, 2048, 3072),
                ("q", q_r, 3072, 4096),
            ):
                pieces = []
                for s0 in range(lo, hi, 1024):
                    s1 = min(s0 + 1024, hi)
                    part = s0 // half
                    pieces.append(
                        (r_dst, s0, s1, raws[(nm, part)], s0 - half * part)
                    )
                cast_queue.extend(pieces)

            def emit_casts(n):
                for _ in range(min(n, len(cast_queue))):
                    r_dst, s0, s1, raw, r0 = cast_queue.pop(0)
                    nc.vector.tensor_scalar_mul(
                        r_dst[:, s0:s1], raw[:, r0 : r0 + (s1 - s0)], QK_PRE
                    )

            # W_h[:, j, :] = [V^T ; ones] s-tile j: [128 s, 65] fp16, built
            # via xbar DMA transpose (t[p, j, c] = v16[c, 128j+p]) in 4
            # pieces of 8 j-tiles so the first PVs aren't blocked on the
            # full transpose.
            ws = []
            for h in range(HPC):
                w_h = singles.tile([128, NJ, 65], dt.float16, tag=f"W{h}")
                nc.vector.memset(w_h[:, :, 64:65], 1.0)
                ws.append(w_h)
            v16 = singles.tile([128, L], dt.float16)
            NPIECE = 4
            PW = L // NPIECE  # 1024 cols -> 8 j-tiles per piece
            tcur = [0]
            PE_TRANSPOSE = bool(os.environ.get("KERNEL_PE_TRANSPOSE"))
            if PE_TRANSPOSE:
                ident = singles.tile([128, 128], dt.float32)
                make_identity(nc, ident[:])

            def emit_transposes():
                p = tcur[0]
                if p >= NPIECE:
                    return
                tcur[0] += 1
                if PE_TRANSPOSE:
                    for h in range(HPC):
                        for j in range(8 * p, 8 * (p + 1)):
                            pt = stp.tile([128, 64], dt.float32, tag="st", name="pt")
                            chain_pe(
                                nc.tensor.transpose(
                                    pt[:],
                                    v_both[
                                        64 * h : 64 * h + 64, 128 * j : 128 * (j + 1)
                                    ],
                                    ident[64 * h : 64 * h + 64, 64 * h : 64 * h + 64],
                                )
                            )
                            nc.vector.tensor_copy(ws[h][:, j, 0:64], pt[:])
                    return
                csl = slice(PW * p, PW * (p + 1))
                nc.vector.tensor_copy(v16[:, csl], v_both[:, csl])
                jsl = slice(8 * p, 8 * (p + 1))
                for h in range(HPC):
                    # xbar writes contiguously (ignores strided out APs):
                    # land in a contiguous scratch, strided-copy into W
                    vt = expp.tile([128, 8, 64], dt.float16, tag="vt", name="vt")
                    nc.scalar.dma_start_transpose(
                        vt[:], v16[64 * h : 64 * h + 64, csl]
                    )
                    nc.vector.tensor_copy(ws[h][:, jsl, 0:64], vt[:])

            def emit_tail(i, h, pv_ps):
                tsl = slice(NT * i, NT * (i + 1))
                # copy PSUM->SBUF first so the pv bank frees immediately
                pv_sb = tails.tile([65, NT], dt.float32, tag="pvsb")
                nc.vector.tensor_copy(pv_sb[:], pv_ps[:])
                den = tails.tile([1, NT], dt.float32, tag="den")
                nc.vector.tensor_copy(den[:], pv_sb[64:65, :])
                d_bc = tails.tile([64, NT], dt.float32, tag="dbc")
                nc.gpsimd.partition_broadcast(d_bc[:], den[:])
                r_bc = tails.tile([64, NT], dt.float32, tag="rbc")
                if os.environ.get("KERNEL_EXACT_RECIP"):
                    nc.vector.reciprocal(r_bc[:], d_bc[:])
                else:
                    nc.vector.reciprocal_approx_fast(out=r_bc[:], in_=d_bc[:])
                o_sb = outp.tile([64, NT], dt.float32, tag="o")
                nc.vector.tensor_mul(o_sb[:], pv_sb[0:64, :], r_bc[:])
                nc.sync.dma_start(o_d[64 * h : 64 * h + 64, tsl], o_sb[:])

            def flush(pend):
                # interleave heads so consecutive PV matmuls alternate PSUM
                # banks (same-bank back-to-back matmuls serialize fully)
                i, j0, cnt, ews, pvs = pend
                for jj in range(cnt):
                    j = j0 + jj
                    esl = slice(NT * jj, NT * (jj + 1))
                    for h in range(HPC):
                        chain_pe(
                            nc.tensor.matmul(
                                pvs[h][:],
                                ws[h][:, j, :],
                                ews[h][:, esl],
                                start=(j == 0),
                                stop=(j == NJ - 1),
                            )
                        )
                if j0 + cnt == NJ:
                    for h in range(HPC):
                        emit_tail(i, h, pvs[h])

            # Global software pipeline over (t-chunk, window): QK(g) and
            # exp(g) are emitted one window ahead of PV(g-1).
            pend = None
            gwin = [0]
            gexp = [0]  # (window, head) exp counter for engine assignment
            for _rep in range(repeat):
                for i in range(TCH):
                    tsl = slice(NT * i, NT * (i + 1))
                    pvs = [
                        pvp.tile([65, NT], dt.float32, tag="pv", name=f"pv{h}")
                        for h in range(HPC)
                    ]
                    for j0, cnt in windows:
                        if cast_queue and gwin[0] in (2, 5, 8, 10, 11, 12, 13):
                            emit_casts(1)
                        gwin[0] += 1
                        sts = [
                            stp.tile(
                                [128, WSZ * NT], dt.float32, tag="st", name=f"st{h}"
                            )
                            for h in range(HPC)
                        ]
                        # interleave heads so row-packed QK pairs overlap on PE
                        for jj in range(cnt):
                            j = j0 + jj
                            for h in range(HPC):
                                hsl = slice(64 * h, 64 * h + 64)
                                chain_pe(
                                    nc.tensor.matmul(
                                        sts[h][:, NT * jj : NT * (jj + 1)],
                                        k_r[hsl, 128 * j : 128 * (j + 1)],
                                        q_r[hsl, tsl],
                                        start=True,
                                        stop=True,
                                        tile_position=(64 * h, 0),
                                    )
                                )
                        if tcur[0] < NPIECE:
                            emit_transposes()
                        ews = []
                        for h in range(HPC):
                            ew = expp.tile([128, WSZ * NT], dt.float16, tag="ew")
                            g = gexp[0]
                            gexp[0] += 1
                            use_dve = (g * DVE_NUM) % DVE_DEN < DVE_NUM
                            if use_dve:
                                nc.vector._custom_dve(
                                    EXP2,
                                    out=ew[:, 0 : cnt * NT].bitcast(dt.int16),
                                    in0=sts[h][:, 0 : cnt * NT],
                                    in1=offt[:].to_broadcast((128, cnt * NT)),
                                    s0=M_MAGIC,
                                    s1=G2,
                                    imm2=G1,
                                )
                            else:
                                nc.scalar.activation(
                                    ew[:, 0 : cnt * NT],
                                    sts[h][:, 0 : cnt * NT],
                                    mybir.ActivationFunctionType.Exp,
                                    bias=actb[:],
                                    scale=ACT_SCALE,
                                )
                            ews.append(ew)
                        if pend is not None:
                            flush(pend)
                        pend = (i, j0, cnt, ews, pvs)
            flush(pend)

    nc.compile()
    return nc


PV_CHAIN = not os.environ.get("KERNEL_PV_UNCHAIN")


def _pv(nc, pv_ps, w_h, pending, chain_pe):
    # Column-split PV: j=0 runs as one M=65 matmul with start=True (sets
    # has_written for the whole tile); later s-tiles split into an M=64
    # channel matmul (col groups 0-1) and an M=1 denominator matmul (col
    # group 2) with disjoint output partitions in the same bank. The two
    # run concurrently on the array and each pair's LDWEIGHTS overlaps
    # the other's stream, so a pair costs ~N cycles instead of the
    # serial LDW+MM of a single M=65 matmul.
    ew, j0, cnt = pending
    for jj in range(cnt):
        j = j0 + jj
        esl = slice(NT * jj, NT * (jj + 1))
        if j == 0:
            bi = nc.tensor.matmul(
                pv_ps[:],
                w_h[:, j, :],
                ew[:, esl],
                start=True,
                stop=False,
                tile_position=(0, 0),
            )
            if PV_CHAIN:
                chain_pe(bi)
            continue
        last = j == NJ - 1
        for c0, cw, pos in ((0, 64, (0, 0)), (64, 1, (0, 64))):
            bi = nc.tensor.matmul(
                pv_ps[c0 : c0 + cw, :],
                w_h[:, j, c0 : c0 + cw],
                ew[:, esl],
                start=False,
                stop=last and (c0 == 64),
                tile_position=pos,
            )
            if PV_CHAIN:
                chain_pe(bi)


def _get_nc(repeat: int = 1):
    if repeat not in _nc_cache:
        _nc_cache[repeat] = _build(repeat)
    return _nc_cache[repeat]


def kernel(qkv: np.ndarray, _repeat: int = 1) -> np.ndarray:
    qkv = np.asarray(qkv)
    bs, width, length = qkv.shape
    assert (bs, width, length) == (BS, 3 * NUM_HEADS * CH, L), qkv.shape
    hw = NUM_HEADS * CH

    nc = _get_nc(_repeat)
    in_maps = []
    for c in range(NCORES):
        qs, ks, vs = [], [], []
        for i in range(HPC):
            bh = c * HPC + i
            b, h = bh // NUM_HEADS, bh % NUM_HEADS
            qs.append(qkv[b, h * CH : (h + 1) * CH, :])
            ks.append(qkv[b, hw + h * CH : hw + (h + 1) * CH, :])
            vs.append(qkv[b, 2 * hw + h * CH : 2 * hw + (h + 1) * CH, :])
        in_maps.append(
            {
                "q": np.ascontiguousarray(np.concatenate(qs, axis=0)),
                "k": np.ascontiguousarray(np.concatenate(ks, axis=0)),
                "v": np.ascontiguousarray(np.concatenate(vs, axis=0)),
            }
        )

    res = run_bass_kernel_spmd(nc, in_maps, list(range(NCORES)))

    out = np.empty((BS, hw, L), np.float32)
    for c in range(NCORES):
        oc = res.results[c]["o"]
        for i in range(HPC):
            bh = c * HPC + i
            b, h = bh // NUM_HEADS, bh % NUM_HEADS
            out[b, h * CH : (h + 1) * CH, :] = oc[i * CH : (i + 1) * CH]
    return out
